# revision 1
# baseline (speedup 1.0000x reference)
"""ExpandedPerformerFeatureMap TRN2 Bass kernel.

Computes out[b,h,l,m] = exp(-||x/d^0.25||^2/2) / sqrt(m) * exp((x/d^0.25) @ W^T)
                      = exp(proj + c) with
    proj = (s*x) @ W^T          (s = d^-0.25, folded into a bf16 cast of x)
    c    = -0.0625 * sum(x^2) - 0.5*ln(m)   (per-row bias, fused into ACT Exp)

Sharding: pure data parallel over rows (b*h*l = 262144) across 8 NeuronCores,
random_feats replicated. No collectives.
"""

import numpy as np

import concourse.bass as bass
import concourse.tile as tile
from concourse import mybir
from concourse.bass import compact_to_ranges
from concourse.bass_utils import run_bass_kernel_spmd
from concourse.masks import make_identity

# Problem constants (hardcoded per harness contract).
B, H, L, D = 4, 16, 4096, 64
M = 256
N_CORES = 8
ROWS = B * H * L                 # 262144
ROWS_PER_CORE = ROWS // N_CORES  # 32768
RPB = 1024                       # rows per block
J = RPB // 128                   # 8 row-groups per block (rows 8p+j on partition p)
T = ROWS_PER_CORE // RPB         # 32 blocks per core

SCALE = float(D) ** -0.25                  # fold into x cast
SSQ_SCALE = -0.5 * float(D) ** -0.5        # -0.0625: scale on sum(x^2)
BIAS_CONST = -0.5 * float(np.log(M))       # -0.5*ln(256)

FP32 = mybir.dt.float32
BF16 = mybir.dt.bfloat16


# --- workarounds for the walrus build in this container ---------------------
# (1) EVENT_SEMAPHORE_RANGE_CLEAR (the Tile-tail bulk semaphore clear) fails
#     codegen ("ISA wrong length"). The NEFF executes once per load here, so
#     skip the clear but keep the DMA drain + semaphore bookkeeping.
# (2) The encoder accepts at most ONE semaphore wait per instruction; Tile
#     attaches several. Move excess waits onto same-engine NoOps inserted
#     right before the owning instruction (identical wait-for-all semantics).


def _clear_and_free_semaphores_no_rangeclear(self, sems):
    if not sems:
        return
    sem_nums = [s.num if hasattr(s, "num") else s for s in sems]
    for sem_range in compact_to_ranges(sem_nums):
        assert self._state.free_isdisjoint(sem_range)
        self.gpsimd.dma_reset(sem_range)
    self._state.prepend_free_semaphores(sem_nums)
    for poison_set in self._tile_sem_poison_stack:
        poison_set.update(sem_nums)


def _drain_and_barrier_trim(self, tick_clock, wait_clock):
    """Tile-tail replacement: drain + ONE barrier. The semaphore RANGE_CLEAR
    (unsupported by this walrus) and the dma_reset + second barrier only
    matter for NEFF re-execution; this NEFF runs once per load."""
    from concourse.vector_clock import ScopedClock

    drain_inst = self.nc.sync.drain()
    wait_clock.add_sem_waits(
        drain_inst.ins, ScopedClock({None: tick_clock.global_clock})
    )
    self.nc.all_engine_barrier()
    popped = self.nc._tile_sem_poison_stack.pop()
    assert popped is self._sem_poison
    sems = list(self.sems.allocated().values())
    sem_nums = [s.num if hasattr(s, "num") else s for s in sems]
    self.nc._state.prepend_free_semaphores(sem_nums)
    for poison_set in self.nc._tile_sem_poison_stack:
        poison_set.update(sem_nums)


def _split_excess_waits(nc):
    n_new = 0
    for func in nc.m.functions:
        for block in func.blocks:
            new_insts = []
            for inst in block.instructions:
                si = getattr(inst, "sync_info", None)
                waits = list(si.on_wait) if si is not None and si.on_wait else []
                if len(waits) > 1:
                    for w in waits[:-1]:
                        n_new += 1
                        nop = mybir.InstNoOp(
                            name=f"{inst.name}-xw{n_new}", ins=[], outs=[]
                        )
                        nop.engine = inst.engine
                        nop.sync_info = mybir.SyncInfo(on_wait=[w], on_update=[])
                        new_insts.append(nop)
                    si.on_wait = [waits[-1]]
                new_insts.append(inst)
            if n_new:
                block.instructions[:] = new_insts
    return n_new


def _build_kernel(nc: bass.Bass):
    x_ap = nc.dram_tensor("x", [ROWS_PER_CORE, D], FP32, kind="ExternalInput").ap()
    w_ap = nc.dram_tensor("w", [2, 128, D], FP32, kind="ExternalInput").ap()
    out_ap = nc.dram_tensor(
        "out", [ROWS_PER_CORE, M], FP32, kind="ExternalOutput"
    ).ap()

    # Two 512-row mini-blocks prime the pipeline (first store enters the DMA
    # ring ~3us earlier), then full 1024-row blocks.
    blocks = [(0, 4), (512, 4)]
    r = 1024
    while r < ROWS_PER_CORE:
        blocks.append((r, J))
        r += 128 * J

    with tile.TileContext(nc) as tc:
        with (
            tc.tile_pool(name="consts", bufs=1) as consts,
            tc.tile_pool(name="xin", bufs=12) as xin_pool,
            tc.tile_pool(name="xbf", bufs=5) as xbf_pool,
            tc.tile_pool(name="sq", bufs=4) as sq_pool,
            tc.tile_pool(name="cbias", bufs=8) as c_pool,
            tc.tile_pool(name="xt", bufs=12) as xt_pool,
            tc.tile_pool(name="outp", bufs=6) as out_pool,
            tc.tile_pool(name="tpp", bufs=3, space="PSUM") as tp_psum,
            tc.tile_pool(name="mmp", bufs=5, space="PSUM") as mm_psum,
        ):
            # --- one-time: identity (bf16) for PE transpose ---
            identity = consts.tile([128, 128], BF16)
            make_identity(nc, identity)

            # --- one-time: W^T [64, 256] bf16 in SBUF ---
            w_raw = consts.tile([128, 2, D], FP32)
            nc.gpsimd.dma_start(
                out=w_raw[:], in_=w_ap.rearrange("h p d -> p h d")
            )
            w_bf = consts.tile([128, 2, D], BF16)
            nc.vector.tensor_copy(w_bf[:], w_raw[:])
            # W^T replicated on partitions 0:64 and 64:128 so both halves of a
            # transposed x-pair (base partition 0 / 64) see a matching rhs.
            wT = consts.tile([128, 2 * 128], BF16)
            for h in range(2):
                ps = tp_psum.tile([D, 128], BF16, tag="tps")
                nc.tensor.transpose(ps[:], w_bf[:, h, :], identity[:])
                nc.vector.tensor_copy(wT[0:D, h * 128 : (h + 1) * 128], ps[:])
            nc.gpsimd.dma_start(out=wT[D : 2 * D, :], in_=wT[0:D, :])

            # --- main loop ---
            for r0, jb in blocks:
                rows = 128 * jb
                x_view = x_ap[r0 : r0 + rows, :].rearrange(
                    "(p j) d -> p j d", j=jb
                )
                out_view = out_ap[r0 : r0 + rows, :].rearrange(
                    "(p j) m -> p j m", j=jb
                )

                x_t = xin_pool.tile([128, jb, D], FP32, tag="x_t")
                nc.sync.dma_start(out=x_t[:], in_=x_view)

                # bf16 cast with s = d^-0.25 folded in
                x_bf = xbf_pool.tile([128, jb, D], BF16, tag="x_bf")
                nc.vector.tensor_scalar_mul(x_bf[:], x_t[:], SCALE)

                # per-row bias c = -0.0625*sum(x^2) - 0.5*ln(m)
                sq = sq_pool.tile([128, jb, D], FP32, tag="sq")
                nc.vector.tensor_mul(sq[:], x_t[:], x_t[:])
                r_t = c_pool.tile([128, jb], FP32, tag="rt")
                nc.vector.tensor_reduce(
                    out=r_t[:], in_=sq[:],
                    axis=mybir.AxisListType.X, op=mybir.AluOpType.add,
                )
                c_t = c_pool.tile([128, jb], FP32, tag="ct")
                nc.vector.tensor_scalar(
                    out=c_t[:], in0=r_t[:],
                    scalar1=SSQ_SCALE, scalar2=BIAS_CONST,
                    op0=mybir.AluOpType.mult, op1=mybir.AluOpType.add,
                )

                # transpose row-group pairs: [128, 2, 64] -> [128(=2 rows x 64 feat), 128]
                xts = []
                for tp in range(jb // 2):
                    ps = tp_psum.tile([128, 128], BF16, tag="tps")
                    nc.tensor.transpose(ps[:], x_bf[:, 2 * tp : 2 * tp + 2, :], identity[:])
                    xt = xt_pool.tile([128, 128], BF16, tag="xt")
                    nc.vector.tensor_copy(xt[:], ps[:])
                    xts.append(xt)

                out_t = out_pool.tile([128, jb, M], FP32, tag="out_t")
                for j in range(jb):
                    mm = mm_psum.tile([128, M], FP32, tag="mm")
                    half = j % 2
                    lhsT = xts[j // 2][half * D : (half + 1) * D, :]
                    rhs = wT[half * D : (half + 1) * D, :]
                    nc.tensor.matmul(mm[:], lhsT, rhs, start=True, stop=True)
                    nc.scalar.activation(
                        out=out_t[:, j, :],
                        in_=mm[:],
                        func=mybir.ActivationFunctionType.Exp,
                        bias=c_t[:, j : j + 1],
                        scale=1.0,
                    )

                nc.sync.dma_start(out=out_view, in_=out_t[:])

    return nc


_NC_CACHE = None


def _get_nc():
    global _NC_CACHE
    if _NC_CACHE is None:
        orig = bass.Bass.clear_and_free_semaphores
        orig_dab = tile.TileContext._drain_and_barrier
        bass.Bass.clear_and_free_semaphores = _clear_and_free_semaphores_no_rangeclear
        tile.TileContext._drain_and_barrier = _drain_and_barrier_trim
        try:
            nc = bass.Bass("TRN2", target_bir_lowering=False, debug=False,
                           num_devices=N_CORES)
            _build_kernel(nc)
        finally:
            bass.Bass.clear_and_free_semaphores = orig
            tile.TileContext._drain_and_barrier = orig_dab
        _split_excess_waits(nc)
        _NC_CACHE = nc
    return _NC_CACHE


def kernel(x: np.ndarray, random_feats: np.ndarray, _trace=False, _tmpdir=None):
    nc = _get_nc()
    xs = np.ascontiguousarray(np.asarray(x), dtype=np.float32).reshape(ROWS, D)
    w = np.ascontiguousarray(np.asarray(random_feats), dtype=np.float32).reshape(
        2, 128, D
    )
    in_maps = []
    for i in range(N_CORES):
        shard = xs[i * ROWS_PER_CORE : (i + 1) * ROWS_PER_CORE]
        in_maps.append({"x": np.ascontiguousarray(shard), "w": w})
    res = run_bass_kernel_spmd(
        nc, in_maps, core_ids=list(range(N_CORES)), trace=_trace, tmpdir=_tmpdir
    )
    out = np.empty((ROWS, M), dtype=np.float32)
    for i in range(N_CORES):
        out[i * ROWS_PER_CORE : (i + 1) * ROWS_PER_CORE] = (
            res.results[i]["out"].reshape(ROWS_PER_CORE, M)
        )
    full = out.reshape(B, H, L, M)
    if _trace:
        return full, res
    return full



# revision 2
# speedup vs baseline: 1.0518x; 1.0518x over previous
"""ExpandedPerformerFeatureMap TRN2 Bass kernel, v4.

out[r, m] = exp(proj[r, m] + c[r]),  proj = xs @ W^T,  xs = x * d^-0.25,
c[r] = -0.5*||xs_r||^2 - ln(16)

v3 vs v2: every matmul has K >= 64 and K=128 matmuls recur, because the
PE activity monitor (HAM) only unthrottles 1.2 -> 2.4 GHz for full-K
work: K=1 rank-1 bias matmuls actively re-throttle it (measured).
 - warm-pump: ~20 K=128 matmuls at kernel start trigger 2.4 GHz while
   the first x chunk loads.
 - bias add is a K=128 matmul: lhsT = c-tile (rows 0/1 = c_even/c_odd,
   rows 2-127 zeroed once per buffer), rhs = parity mask [128, 512].
 - per tile one PSUM accumulation group: projE(start) -> projO -> bias.
Everything else as v2: fp16 host-packed paired-transpose input, fp16
output (host upcast), sumsq via sel matmul, bias-free big-span ACT Exp.
"""

import numpy as np

import concourse.bass as bass
import concourse.tile as tile
from concourse import mybir
from concourse.bass import compact_to_ranges
from concourse.bass_utils import run_bass_kernel_spmd

B, H, L, D = 4, 16, 4096, 64
M = 256
N_CORES = 8
ROWS = B * H * L
ROWS_PER_CORE = ROWS // N_CORES    # 32768
PAIRS = ROWS_PER_CORE // 2         # 16384
NT = PAIRS // 128                  # 128 tiles of 128 pairs
LCHUNK = 2048                      # pairs per input DMA
PCH = 3                            # tiles per PSUM/ACT chunk (3 banks)
CGRP = 4                           # tiles per sumsq matmul group
CSB_BUFS = 6
N_PUMP = 14

SCALE = float(D) ** -0.25
NEG_LN16 = -float(np.log(M)) / 2.0

FP32 = mybir.dt.float32
F16 = mybir.dt.float16


# --- walrus build workarounds (see kernel2) ---------------------------------

def _clear_and_free_semaphores_no_rangeclear(self, sems):
    if not sems:
        return
    sem_nums = [s.num if hasattr(s, "num") else s for s in sems]
    for sem_range in compact_to_ranges(sem_nums):
        assert self._state.free_isdisjoint(sem_range)
        self.gpsimd.dma_reset(sem_range)
    self._state.prepend_free_semaphores(sem_nums)
    for poison_set in self._tile_sem_poison_stack:
        poison_set.update(sem_nums)


def _drain_and_barrier_trim(self, tick_clock, wait_clock):
    from concourse.vector_clock import ScopedClock

    drain_inst = self.nc.sync.drain()
    wait_clock.add_sem_waits(
        drain_inst.ins, ScopedClock({None: tick_clock.global_clock})
    )
    self.nc.all_engine_barrier()
    popped = self.nc._tile_sem_poison_stack.pop()
    assert popped is self._sem_poison
    sems = list(self.sems.allocated().values())
    sem_nums = [s.num if hasattr(s, "num") else s for s in sems]
    self.nc._state.prepend_free_semaphores(sem_nums)
    for poison_set in self.nc._tile_sem_poison_stack:
        poison_set.update(sem_nums)


def _split_excess_waits(nc):
    n_new = 0
    for func in nc.m.functions:
        for block in func.blocks:
            new_insts = []
            for inst in block.instructions:
                si = getattr(inst, "sync_info", None)
                waits = list(si.on_wait) if si is not None and si.on_wait else []
                if len(waits) > 1:
                    for w in waits[:-1]:
                        n_new += 1
                        nop = mybir.InstNoOp(
                            name=f"{inst.name}-xw{n_new}", ins=[], outs=[]
                        )
                        nop.engine = inst.engine
                        nop.sync_info = mybir.SyncInfo(on_wait=[w], on_update=[])
                        new_insts.append(nop)
                    si.on_wait = [waits[-1]]
                new_insts.append(inst)
            if n_new:
                block.instructions[:] = new_insts
    return n_new


def _build_kernel(nc: bass.Bass):
    x_ap = nc.dram_tensor("x2t", [128, PAIRS], F16, kind="ExternalInput").ap()
    cst_ap = nc.dram_tensor("cst", [128, 2 * M + 2], F16, kind="ExternalInput").ap()
    # out layout [partition, tile, (parity, m)]: each partition's stores are
    # contiguous in HBM (strided 1KB-chunk stores measured ~270 GB/s vs ~350
    # contiguous); host restores row order with one transpose.
    out_ap = nc.dram_tensor(
        "out", [128, NT, 2 * M], F16, kind="ExternalOutput"
    ).ap()

    n_chunks = (NT + PCH - 1) // PCH

    with tile.TileContext(nc) as tc:
        with (
            tc.tile_pool(name="consts", bufs=1) as consts,
            tc.tile_pool(name="xin", bufs=4) as xin_pool,
            tc.tile_pool(name="xsq", bufs=2) as xsq_pool,
            tc.tile_pool(name="outp", bufs=4) as out_pool,
            tc.tile_pool(name="expc", bufs=3) as expc_pool,
            tc.tile_pool(name="psproj", bufs=2, space="PSUM") as ps_proj,
            tc.tile_pool(name="psc", bufs=2, space="PSUM") as ps_c,
        ):
            # pump source available at t~0 (no DMA dependency)
            pump_src = consts.tile([128, M], F16)
            nc.vector.memset(pump_src[:], 0.125)
            ln16_sb = consts.tile([128, 1], FP32)
            nc.vector.memset(ln16_sb[:], NEG_LN16)

            # --- one-time constants, one DMA: w | mask2 | sel ---
            cst_sb = consts.tile([128, 2 * M + 2], F16)
            nc.sync.dma_start(out=cst_sb[:], in_=cst_ap)
            w_sb = cst_sb[:, 0 : 2 * M]
            selp_sb = cst_sb[:, 2 * M : 2 * M + 2]

            # warm the ACT exp table (runs off pump_src, no DMA wait)
            warm = consts.tile([1, M], F16)
            nc.scalar.activation(
                out=warm[:], in_=pump_src[0:1, :],
                func=mybir.ActivationFunctionType.Exp,
            )

            # Prefetch the first two x chunks (and their squares/sumsq
            # inputs) before the warm-pump so the pump covers their latency
            # and the PE never idles at pipeline start.
            pre_loads = []
            for lc0 in range(2):
                xt0 = xin_pool.tile([128, LCHUNK], F16, tag="xt", name=f"xt_pre{lc0}")
                nc.sync.dma_start(
                    out=xt0[:], in_=x_ap[:, lc0 * LCHUNK : (lc0 + 1) * LCHUNK]
                )
                xq0 = xsq_pool.tile([128, LCHUNK], F16, tag="xq", name=f"xq_pre{lc0}")
                nc.vector.tensor_mul(xq0[:], xt0[:], xt0[:])
                pre_loads.append((xt0, xq0))

            # HAM warm-pump: K=128 matmuls push the PE to 2.4 GHz while the
            # first x chunks load. Spread across banks to avoid WAW waits.
            wp0 = ps_proj.tile([128, PCH, 2, M], FP32, tag="proj")
            wp1 = ps_proj.tile([128, PCH, 2, M], FP32, tag="proj")
            for wi in range(N_PUMP):
                wt = (wp0, wp1)[(wi // PCH) % 2]
                nc.tensor.matmul(
                    wt[:, wi % PCH, (wi // (2 * PCH)) % 2, :],
                    pump_src[:, 0:128], pump_src[:],
                    start=True, stop=True,
                )

            TPL = LCHUNK // 128
            GPL = TPL // CGRP
            xts = [None] * (NT // TPL)
            cs = [None] * (NT // TPL)
            n_c_alloc = 0

            cur_osb = None
            osb_base = 0

            for pc in range(n_chunks):
                t0 = pc * PCH
                nt = min(PCH, NT - t0)
                ps = ps_proj.tile([128, PCH, 2, M], FP32, tag="proj")

                for j in range(nt):
                    t = t0 + j
                    lc, ti = divmod(t, TPL)

                    if xts[lc] is None:
                        if lc < 2:
                            xt, xq = pre_loads[lc]
                        else:
                            xt = xin_pool.tile([128, LCHUNK], F16, tag="xt")
                            nc.sync.dma_start(
                                out=xt[:], in_=x_ap[:, lc * LCHUNK : (lc + 1) * LCHUNK]
                            )
                            xq = xsq_pool.tile([128, LCHUNK], F16, tag="xq")
                            nc.vector.tensor_mul(xq[:], xt[:], xt[:])
                        xts[lc] = xt
                        # per-tile sumsq matmuls into pair-partition PSUM,
                        # then one ACT computes exp(-0.5*ss - ln16) for the
                        # whole load chunk via the activation free-affine.
                        psc = ps_c.tile([128, TPL, 2], FP32, tag="psc")
                        for u in range(TPL):
                            nc.tensor.matmul(
                                psc[:, u, :],
                                xq[:, u * 128 : (u + 1) * 128],
                                selp_sb[:],
                                start=True, stop=True,
                            )
                        expc = expc_pool.tile([128, TPL, 2], FP32, tag="expc")
                        nc.scalar.activation(
                            out=expc[:], in_=psc[:],
                            func=mybir.ActivationFunctionType.Exp,
                            bias=ln16_sb[:], scale=-0.5,
                        )
                        cs[lc] = expc

                    xt = xts[lc]

                    # One K=128 N=512 proj matmul per tile (block-diagonal
                    # W covers both parities). All-K=128 keeps the PE
                    # activity monitor at 2.4 GHz.
                    nc.tensor.matmul(
                        ps[:, j, :, :],
                        xt[:, ti * 128 : (ti + 1) * 128],
                        w_sb[:],
                        start=True, stop=True,
                    )

                if pc % 2 == 0:
                    cur_osb = out_pool.tile([128, 2, PCH, 2 * M], F16, tag="osb")
                    osb_base = t0
                    osb_stored = 0

                nc.scalar.activation(
                    out=cur_osb[:, pc % 2, 0:nt, :],
                    in_=ps[:, 0:nt, :, :],
                    func=mybir.ActivationFunctionType.Exp,
                )
                # multiply exp(proj) by exp(c): one tensor_scalar per
                # (tile, parity) slice with a per-partition f32 scalar AP
                for j in range(nt):
                    lc0, u0 = divmod(t0 + j, TPL)
                    for q in range(2):
                        sl = cur_osb[:, pc % 2, j, q * M : (q + 1) * M]
                        nc.vector.tensor_scalar(
                            out=sl, in0=sl,
                            scalar1=cs[lc0][:, u0, q : q + 1], scalar2=None,
                            op0=mybir.AluOpType.mult,
                        )

                last = pc == n_chunks - 1
                if pc % 2 == 1 or last or pc >= n_chunks - 3:
                    ntiles = t0 + nt - osb_base
                    nc.sync.dma_start(
                        out=out_ap[:, osb_base + osb_stored : osb_base + ntiles, :],
                        in_=cur_osb[:].rearrange("p a b c -> p (a b) c")[
                            :, osb_stored:ntiles, :
                        ],
                    )
                    osb_stored = ntiles

    return nc


_NC_CACHE = None


def _get_nc():
    global _NC_CACHE
    if _NC_CACHE is None:
        orig = bass.Bass.clear_and_free_semaphores
        orig_dab = tile.TileContext._drain_and_barrier
        bass.Bass.clear_and_free_semaphores = _clear_and_free_semaphores_no_rangeclear
        tile.TileContext._drain_and_barrier = _drain_and_barrier_trim
        try:
            nc = bass.Bass("TRN2", target_bir_lowering=False, debug=False,
                           num_devices=N_CORES)
            _build_kernel(nc)
        finally:
            bass.Bass.clear_and_free_semaphores = orig
            tile.TileContext._drain_and_barrier = orig_dab
        _split_excess_waits(nc)
        _NC_CACHE = nc
    return _NC_CACHE


def kernel(x: np.ndarray, random_feats: np.ndarray, _trace=False, _tmpdir=None):
    nc = _get_nc()
    xs = np.asarray(x, dtype=np.float32).reshape(ROWS, D) * np.float32(SCALE)
    xs16 = xs.astype(np.float16)
    w16 = np.asarray(random_feats, dtype=np.float32).T.astype(np.float16)  # (64, 256)
    cst = np.zeros((128, 2 * M + 2), dtype=np.float16)
    cst[0:64, 0:M] = w16                      # w block-diag
    cst[64:128, M : 2 * M] = w16
    cst[0:64, 2 * M] = 1.0                    # parity select (sumsq)
    cst[64:128, 2 * M + 1] = 1.0

    in_maps = []
    for i in range(N_CORES):
        shard = xs16[i * ROWS_PER_CORE : (i + 1) * ROWS_PER_CORE]
        x2t = np.ascontiguousarray(
            shard.reshape(PAIRS, 2, D).transpose(1, 2, 0).reshape(128, PAIRS)
        )
        in_maps.append({"x2t": x2t, "cst": cst})

    res = run_bass_kernel_spmd(
        nc, in_maps, core_ids=list(range(N_CORES)), trace=_trace, tmpdir=_tmpdir
    )
    out = np.empty((ROWS, M), dtype=np.float32)
    for i in range(N_CORES):
        # [128 part, NT, 512] -> pair p = t*128 + i -> natural rows
        oc = res.results[i]["out"].reshape(128, NT, 2 * M).transpose(1, 0, 2)
        out[i * ROWS_PER_CORE : (i + 1) * ROWS_PER_CORE] = (
            np.ascontiguousarray(oc).reshape(ROWS_PER_CORE, M).astype(np.float32)
        )
    full = out.reshape(B, H, L, M)
    if _trace:
        return full, res
    return full
